# revision 1
# baseline (speedup 1.0000x reference)
"""Trainium2 Bass kernel for 16-head causal self-attention (KaplanAttention).

Problem: x [2, 2048, 1024], torch-style weights W_q/W_k/W_v/W_o [1024, 1024].
  q/k/v = (x @ W.T) split into 16 heads of 64; causal softmax(q k^T / 8) @ v;
  concat heads; out = attn_out @ W_o.T.

Sharding (8 cores): core c handles batch b = c // 4 and head group g = c % 4
(heads 4g..4g+3). Each core computes its 4 heads' attention output and a
partial output projection against the matching 256 columns of W_o; the host
sums the 4 partials per batch (the "all-reduce" of the row-sharded W_o).

Per-core layout (everything transposed on host so the PE contraction dim is
always the partition dim; all matmul operands fp16, accumulation fp32):
  xT  [1024, 2048] = x[b].T                      e on partitions
  wqT/wkT/wvT [1024, 256] = W[256g:256g+256].T   e on partitions
  woT [256, 1024] = W_o[:, 256g:256g+256].T      d on partitions
  QT/KT [128, 2, 2048]: head pair hp, head h at partitions 64*(h%2)
  V     [128, 16, 4, 65]: j-chunk k, head h -> [V_h | ones] (ones col gives
        the softmax denominator for free from the same matmul)
  scores computed transposed: S^T[j, s] tiles [128 j, <=512 s]; exp on ACT
  (scale=1/8 fused); causal handled by only computing s >= 128*jt and a
  {0,1} upper-tri mask on diagonal 128x128 blocks.
  AV: psum [65, s] accumulates [V|1]^T @ U^T; row 64 = Z_s. Normalize via
  reciprocal + gpsimd partition_broadcast + tensor_mul.
  Final: partial[s, m] from lhsT = normalized out^T, rhs = woT chunks.
"""

import numpy as np

from concourse import bass_utils, mybir, tile
from concourse import bacc

S = 2048
D = 1024
HPC = 4        # heads per core
DK = 64
DC = HPC * DK  # 256 d-columns per core
NCORES = 8
EC = D // 128  # 8 e-chunks
NJT = S // 128  # 16 j-tiles
NST = S // 512  # 4 s 512-tiles

FP16 = mybir.dt.float16
FP32 = mybir.dt.float32

# ragged offsets of each j-tile's region inside a UT row [128, sum(2048-128*jt)]
UT_OFF = []
_o = 0
for _jt in range(NJT):
    UT_OFF.append(_o)
    _o += S - 128 * _jt
UT_COLS = _o  # 17408


def _build(reps=1):
    nc = bacc.Bacc("TRN2", target_bir_lowering=False, debug=False)

    xT_d = nc.dram_tensor("xT", [D, S], FP16, kind="ExternalInput")
    wq_d = nc.dram_tensor("wqT", [D, DC], FP16, kind="ExternalInput")
    wk_d = nc.dram_tensor("wkT", [D, DC], FP16, kind="ExternalInput")
    wv_d = nc.dram_tensor("wvT", [D, DC], FP16, kind="ExternalInput")
    wo_d = nc.dram_tensor("woT", [DC, D], FP16, kind="ExternalInput")
    mask_d = nc.dram_tensor("mask", [128, 128], FP16, kind="ExternalInput")
    out_d = nc.dram_tensor("out", [S, D], FP32, kind="ExternalOutput")

    with tile.TileContext(nc) as tc:
        with (
            tc.tile_pool(name="const", bufs=1) as const,
            tc.tile_pool(name="work", bufs=1) as work,
            tc.tile_pool(name="ut", bufs=1) as utp,
            tc.tile_pool(name="outs", bufs=3) as outs,
            tc.tile_pool(name="norm", bufs=4) as normp,
            tc.tile_pool(name="psA", bufs=2, space="PSUM") as psA,
            tc.tile_pool(name="psS", bufs=1, space="PSUM") as psS,
            tc.tile_pool(name="psV", bufs=2, space="PSUM") as psV,
        ):
          for _rep in range(reps):
            # ---- load inputs ----
            xT = const.tile([128, EC, S], FP16)
            for c in range(EC):
                nc.sync.dma_start(out=xT[:, c, :], in_=xT_d[128 * c : 128 * (c + 1), :])
            wq = const.tile([128, EC, DC], FP16)
            wk = const.tile([128, EC, DC], FP16)
            wv = const.tile([128, EC, DC], FP16)
            for w_t, w_dr in ((wq, wq_d), (wk, wk_d), (wv, wv_d)):
                nc.sync.dma_start(
                    out=w_t, in_=w_dr.rearrange("(c p) d -> p c d", p=128)
                )
            wo = const.tile([128, 2, D], FP16)
            nc.sync.dma_start(out=wo, in_=wo_d.rearrange("(c p) d -> p c d", p=128))
            mask = const.tile([128, 128], FP16)
            nc.sync.dma_start(out=mask, in_=mask_d[:, :])

            # ---- projections ----
            QT = work.tile([128, 2, S], FP16)
            KT = work.tile([128, 2, S], FP16)
            for w_t, dst in ((wq, QT), (wk, KT)):
                for hp in range(2):
                    for st in range(NST):
                        ps = psV.tile([128, 512], FP32, tag="proj")
                        for c in range(EC):
                            nc.tensor.matmul(
                                ps,
                                w_t[:, c, 128 * hp : 128 * (hp + 1)],
                                xT[:, c, 512 * st : 512 * (st + 1)],
                                start=(c == 0),
                                stop=(c == EC - 1),
                            )
                        nc.vector.tensor_copy(
                            out=dst[:, hp, 512 * st : 512 * (st + 1)], in_=ps
                        )

            V = work.tile([128, NJT, HPC, 65], FP16)
            nc.vector.memset(V[:, :, :, 64:65], 1.0)
            for jt in range(NJT):
                ps = psV.tile([128, 512], FP32, tag="proj")
                psd = ps[:, 0:DC]
                for c in range(EC):
                    nc.tensor.matmul(
                        psd,
                        xT[:, c, 128 * jt : 128 * (jt + 1)],
                        wv[:, c, :],
                        start=(c == 0),
                        stop=(c == EC - 1),
                    )
                nc.vector.tensor_copy(
                    out=V[:, jt, :, 0:64],
                    in_=psd.rearrange("p (h d) -> p h d", h=HPC),
                )

            # ---- attention + output projection ----
            outTn = work.tile([128, 2, S], FP16)  # normalized out^T, pair-stacked

            for hp in range(2):
                # scores^T + exp, both heads of the pair in one ACT op
                UT = utp.tile([128, 2, UT_COLS], FP16, tag="ut")
                for jt in range(NJT):
                    s0 = 128 * jt
                    pos = s0
                    while pos < S:
                        cn = min(1024, S - pos)
                        ps = psS.tile([128, 2, 1024], FP32, tag="score")
                        for hi in range(2):
                            ho = 64 * hi
                            for half in range(0, cn, 512):
                                hn = min(512, cn - half)
                                nc.tensor.matmul(
                                    ps[:, hi, half : half + hn],
                                    KT[ho : ho + 64, hp, s0 : s0 + 128],
                                    QT[ho : ho + 64, hp, pos + half : pos + half + hn],
                                    start=True,
                                    stop=True,
                                )
                        uo = UT_OFF[jt] + (pos - s0)
                        nc.scalar.activation(
                            out=UT[:, :, uo : uo + cn],
                            in_=ps[:, :, 0:cn],
                            func=mybir.ActivationFunctionType.Exp,
                            scale=0.125,
                        )
                        pos += cn
                    # causal mask on the diagonal 128-block
                    for hi in range(2):
                        nc.vector.tensor_mul(
                            UT[:, hi, UT_OFF[jt] : UT_OFF[jt] + 128],
                            UT[:, hi, UT_OFF[jt] : UT_OFF[jt] + 128],
                            mask,
                        )

                # AV + normalize, per head of the pair
                for hi in range(2):
                    h = 2 * hp + hi
                    ho = 64 * hi
                    for t in range(NST):
                        psa = psA.tile([65, 512], FP32, tag="av")
                        kmax = 4 * t + 4
                        for k in range(kmax):
                            off = max(0, 128 * k - 512 * t)
                            n = 512 - off
                            uo = UT_OFF[k] + (512 * t + off - 128 * k)
                            nc.tensor.matmul(
                                psa[:, off : off + n],
                                V[:, k, h, :],
                                UT[:, hi, uo : uo + n],
                                start=(k == 0),
                                stop=(k == kmax - 1),
                            )
                        zr = normp.tile([1, 512], FP32, tag="zrow")
                        nc.vector.reciprocal(out=zr, in_=psa[64:65, :])
                        zb = normp.tile([64, 512], FP32, tag="zb")
                        nc.gpsimd.partition_broadcast(zb, zr)
                        nc.vector.tensor_mul(
                            outTn[ho : ho + 64, hp, 512 * t : 512 * (t + 1)],
                            psa[0:64, :],
                            zb,
                        )

            # ---- final projection: partial[s, m] ----
            for st in range(NJT):
                ob = outs.tile([128, D], FP32, tag="ob")
                for mt in range(2):
                    psf = psV.tile([128, 512], FP32, tag="proj")
                    for hp in range(2):
                        nc.tensor.matmul(
                            psf,
                            outTn[:, hp, 128 * st : 128 * (st + 1)],
                            wo[:, hp, 512 * mt : 512 * (mt + 1)],
                            start=(hp == 0),
                            stop=(hp == 1),
                        )
                    nc.vector.tensor_copy(out=ob[:, 512 * mt : 512 * (mt + 1)], in_=psf)
                nc.sync.dma_start(out=out_d[128 * st : 128 * (st + 1), :], in_=ob)

    nc.compile()
    return nc


_NC = None


def _prep_in_maps(x, W_q, W_k, W_v, W_o):
    x = np.asarray(x, dtype=np.float32)
    W_q = np.asarray(W_q, dtype=np.float32)
    W_k = np.asarray(W_k, dtype=np.float32)
    W_v = np.asarray(W_v, dtype=np.float32)
    W_o = np.asarray(W_o, dtype=np.float32)
    mask01 = np.triu(np.ones((128, 128), dtype=np.float16))
    in_maps = []
    for c in range(NCORES):
        b, g = divmod(c, 4)
        cols = slice(DC * g, DC * (g + 1))
        in_maps.append(
            {
                "xT": np.ascontiguousarray(x[b].T).astype(np.float16),
                "wqT": np.ascontiguousarray(W_q[cols, :].T).astype(np.float16),
                "wkT": np.ascontiguousarray(W_k[cols, :].T).astype(np.float16),
                "wvT": np.ascontiguousarray(W_v[cols, :].T).astype(np.float16),
                "woT": np.ascontiguousarray(W_o[:, cols].T).astype(np.float16),
                "mask": mask01,
            }
        )
    return in_maps


def _run(x, W_q, W_k, W_v, W_o, **spmd_kwargs):
    global _NC
    if _NC is None:
        _NC = _build()
    in_maps = _prep_in_maps(x, W_q, W_k, W_v, W_o)
    res = bass_utils.run_bass_kernel_spmd(
        _NC, in_maps, core_ids=list(range(NCORES)), **spmd_kwargs
    )
    parts = [res.results[c]["out"] for c in range(NCORES)]
    out = np.empty((2, S, D), dtype=np.float32)
    for b in range(2):
        out[b] = parts[4 * b] + parts[4 * b + 1] + parts[4 * b + 2] + parts[4 * b + 3]
    return out, res


def kernel(x, W_q, W_k, W_v, W_o):
    out, _ = _run(x, W_q, W_k, W_v, W_o)
    return out



# revision 2
# speedup vs baseline: 1.4802x; 1.4802x over previous
"""Trainium2 Bass kernel v2 for 16-head causal self-attention (KaplanAttention).

Problem: x [2, 2048, 1024], torch-style weights W_q/W_k/W_v/W_o [1024, 1024].
  q/k/v = (x @ W.T) split into 16 heads of 64; causal softmax(q k^T / 8) @ v;
  concat heads; out = attn_out @ W_o.T.

Sharding (8 cores): core c handles batch b = c // 4 and head group g = c % 4
(heads 4g..4g+3). Each core computes its 4 heads' attention output and a
partial output projection against the matching 256 columns of W_o; the host
sums the 4 partials per batch (the "all-reduce" of the row-sharded W_o).

v2 structure (vs v1): s-major streaming attention. Per head-pair hp and
512-wide s-window st:
  - scores^T per j-tile: U^T[j, 2 hi, s] = exp(K^T q / 8), fp16, only the
    causal span; triangular {0,1} mask on the diagonal 128-block (DVE).
  - AV flipped: po[s, 2, 65] += U_chunk^T.T @ [V_h | 1] per k-tile, so the
    softmax denominator Z lands as a per-PARTITION column (po[:, hi, 64]) —
    reciprocal is a cheap [128, 2, 1] DVE op (v1 burned 53us on [1, 512]
    row reciprocals) and normalization is a per-partition tensor_scalar.
  - PE transpose (128x128, both heads at once) brings normalized output back
    to [d, s] orientation for the final projection; transposes are deferred
    one st so their DVE-latency never stalls the PE queue.
All matmul operands fp16, accumulation fp32.
"""

import numpy as np

from concourse import bass_utils, mybir, tile
from concourse import bacc

S = 2048
D = 1024
HPC = 4        # heads per core
DK = 64
DC = HPC * DK  # 256 d-columns per core
NCORES = 8
EC = D // 128  # 8 e-chunks
NJT = S // 128  # 16 j-tiles
NST = S // 512  # 4 s-windows of 512

FP16 = mybir.dt.float16
FP32 = mybir.dt.float32


def _build():
    nc = bacc.Bacc("TRN2", target_bir_lowering=False, debug=False)

    xT_d = nc.dram_tensor("xT", [D, S], FP16, kind="ExternalInput")
    wq_d = nc.dram_tensor("wqT", [D, DC], FP16, kind="ExternalInput")
    wk_d = nc.dram_tensor("wkT", [D, DC], FP16, kind="ExternalInput")
    wv_d = nc.dram_tensor("wvT", [D, DC], FP16, kind="ExternalInput")
    wo_d = nc.dram_tensor("woT", [DC, D], FP16, kind="ExternalInput")
    mask_d = nc.dram_tensor("mask", [128, 2 * 128], FP16, kind="ExternalInput")
    id_d = nc.dram_tensor("ident", [128, 128], FP16, kind="ExternalInput")
    out_d = nc.dram_tensor("out", [S, D], FP32, kind="ExternalOutput")

    with tile.TileContext(nc) as tc:
        with (
            tc.tile_pool(name="const", bufs=1) as const,
            tc.tile_pool(name="work", bufs=1) as work,
            tc.tile_pool(name="upool", bufs=2) as upool,
            tc.tile_pool(name="ospool", bufs=10) as ospool,
            tc.tile_pool(name="zpool", bufs=8) as zpool,
            tc.tile_pool(name="obpool", bufs=3) as obpool,
            tc.tile_pool(name="psBig", bufs=2, space="PSUM") as psBig,
            tc.tile_pool(name="psPo", bufs=3, space="PSUM") as psPo,
            tc.tile_pool(name="psPt", bufs=1, space="PSUM") as psPt,
        ):
            # ---- load inputs ----
            xT = const.tile([128, EC, S], FP16)
            for c in range(EC):
                nc.sync.dma_start(out=xT[:, c, :], in_=xT_d[128 * c : 128 * (c + 1), :])
            wq = const.tile([128, EC, DC], FP16)
            wk = const.tile([128, EC, DC], FP16)
            wv = const.tile([128, EC, DC], FP16)
            for w_t, w_dr in ((wq, wq_d), (wk, wk_d), (wv, wv_d)):
                nc.sync.dma_start(
                    out=w_t, in_=w_dr.rearrange("(c p) d -> p c d", p=128)
                )
            wo = const.tile([128, 2, D], FP16)
            nc.sync.dma_start(out=wo, in_=wo_d.rearrange("(c p) d -> p c d", p=128))
            maskD = const.tile([128, 2, 128], FP16)
            nc.sync.dma_start(
                out=maskD, in_=mask_d.rearrange("p (t c) -> p t c", t=2)
            )
            ident = const.tile([128, 128], FP16)
            nc.sync.dma_start(out=ident, in_=id_d[:, :])

            # ---- projections ----
            QT = work.tile([128, 2, S], FP16)
            KT = work.tile([128, 2, S], FP16)
            for w_t, dst in ((wq, QT), (wk, KT)):
                for hp in range(2):
                    for st in range(NST):
                        ps = psBig.tile([128, 2, 512], FP32, tag="big")
                        for c in range(EC):
                            nc.tensor.matmul(
                                ps[:, 0, :],
                                w_t[:, c, 128 * hp : 128 * (hp + 1)],
                                xT[:, c, 512 * st : 512 * (st + 1)],
                                start=(c == 0),
                                stop=(c == EC - 1),
                            )
                        nc.vector.tensor_copy(
                            out=dst[:, hp, 512 * st : 512 * (st + 1)], in_=ps[:, 0, :]
                        )

            V = work.tile([128, NJT, HPC, 65], FP16)
            nc.vector.memset(V[:, :, :, 64:65], 1.0)
            for jt in range(NJT):
                ps = psBig.tile([128, 2, 512], FP32, tag="big")
                psd = ps[:, 0, 0:DC]
                for c in range(EC):
                    nc.tensor.matmul(
                        psd,
                        xT[:, c, 128 * jt : 128 * (jt + 1)],
                        wv[:, c, :],
                        start=(c == 0),
                        stop=(c == EC - 1),
                    )
                nc.vector.tensor_copy(
                    out=V[:, jt, :, 0:64],
                    in_=psd.rearrange("p (h d) -> p h d", h=HPC),
                )

            # ---- attention ----
            outTn = work.tile([128, 2, S], FP16)  # [d-of-pair, hp, s], normalized

            # deferred PE transposes: (hp, sb, os_tile) from the previous st
            pending_t = []

            def emit_transpose(hp, sb, os_t):
                pt = psPt.tile([128, 128], FP16, tag="pt")
                nc.tensor.transpose(pt, os_t, ident)
                nc.vector.tensor_copy(
                    out=outTn[:, hp, 128 * sb : 128 * (sb + 1)], in_=pt
                )

            for hp in range(2):
                for st in range(NST):
                    Ut = upool.tile([128, 2, NJT, 512], FP16, tag="U")
                    for jt in range(4 * st + 4):
                        off = max(0, 128 * jt - 512 * st)
                        n = 512 - off
                        ps = psBig.tile([128, 2, 512], FP32, tag="big")
                        for hi in range(2):
                            ho = 64 * hi
                            nc.tensor.matmul(
                                ps[:, hi, 0:n],
                                KT[ho : ho + 64, hp, 128 * jt : 128 * (jt + 1)],
                                QT[ho : ho + 64, hp, 512 * st + off : 512 * (st + 1)],
                                start=True,
                                stop=True,
                            )
                        nc.scalar.activation(
                            out=Ut[:, :, jt, off : off + n],
                            in_=ps[:, :, 0:n],
                            func=mybir.ActivationFunctionType.Exp,
                            scale=0.125,
                        )
                        if jt >= 4 * st:  # diagonal 128-block: causal mask
                            nc.vector.tensor_mul(
                                Ut[:, :, jt, off : off + 128],
                                Ut[:, :, jt, off : off + 128],
                                maskD,
                            )
                    for sbl in range(4):
                        sb = 4 * st + sbl
                        po = psPo.tile([128, 2, 65], FP32, tag="po")
                        for hi in range(2):
                            for k in range(sb + 1):
                                nc.tensor.matmul(
                                    po[:, hi, :],
                                    Ut[:, hi, k, 128 * sbl : 128 * (sbl + 1)],
                                    V[:, k, 2 * hp + hi, :],
                                    start=(k == 0),
                                    stop=(k == sb),
                                )
                        zr = zpool.tile([128, 2, 1], FP32, tag="zr")
                        nc.vector.reciprocal(out=zr, in_=po[:, :, 64:65])
                        os_t = ospool.tile([128, 2, DK], FP16, tag="os")
                        for hi in range(2):
                            nc.vector.tensor_scalar_mul(
                                os_t[:, hi, :], po[:, hi, 0:64], zr[:, hi, :]
                            )
                        pending_t.append((hp, sb, os_t))
                        # interleave one deferred transpose (from the prior st)
                        # behind this AV chain so its DVE latency is hidden
                        if len(pending_t) > 4:
                            emit_transpose(*pending_t.pop(0))

            # ---- final projection: partial[s, m], interleaved with the
            # last transposes (final sb 0..11 doesn't depend on them) ----
            def emit_final(sb):
                psf = psBig.tile([128, 2, 512], FP32, tag="big")
                for mt in range(2):
                    for hp in range(2):
                        nc.tensor.matmul(
                            psf[:, mt, :],
                            outTn[:, hp, 128 * sb : 128 * (sb + 1)],
                            wo[:, hp, 512 * mt : 512 * (mt + 1)],
                            start=(hp == 0),
                            stop=(hp == 1),
                        )
                ob = obpool.tile([128, 2, 512], FP32, tag="ob")
                nc.vector.tensor_copy(out=ob, in_=psf)
                nc.sync.dma_start(
                    out=out_d[128 * sb : 128 * (sb + 1), :].rearrange(
                        "p (t c) -> p t c", t=2
                    ),
                    in_=ob,
                )

            for sb in range(12):
                emit_final(sb)
                if pending_t and sb >= 7:
                    emit_transpose(*pending_t.pop(0))
            while pending_t:
                emit_transpose(*pending_t.pop(0))
            for sb in range(12, 16):
                emit_final(sb)

    nc.compile()
    return nc


_NC = None


def _prep_in_maps(x, W_q, W_k, W_v, W_o):
    x = np.asarray(x, dtype=np.float32)
    W_q = np.asarray(W_q, dtype=np.float32)
    W_k = np.asarray(W_k, dtype=np.float32)
    W_v = np.asarray(W_v, dtype=np.float32)
    W_o = np.asarray(W_o, dtype=np.float32)
    mask01 = np.triu(np.ones((128, 128), dtype=np.float16))
    mask2 = np.concatenate([mask01, mask01], axis=1)
    ident = np.eye(128, dtype=np.float16)
    in_maps = []
    for c in range(NCORES):
        b, g = divmod(c, 4)
        cols = slice(DC * g, DC * (g + 1))
        in_maps.append(
            {
                "xT": np.ascontiguousarray(x[b].T).astype(np.float16),
                "wqT": np.ascontiguousarray(W_q[cols, :].T).astype(np.float16),
                "wkT": np.ascontiguousarray(W_k[cols, :].T).astype(np.float16),
                "wvT": np.ascontiguousarray(W_v[cols, :].T).astype(np.float16),
                "woT": np.ascontiguousarray(W_o[:, cols].T).astype(np.float16),
                "mask": mask2,
                "ident": ident,
            }
        )
    return in_maps


def _run(x, W_q, W_k, W_v, W_o, **spmd_kwargs):
    global _NC
    if _NC is None:
        _NC = _build()
    in_maps = _prep_in_maps(x, W_q, W_k, W_v, W_o)
    res = bass_utils.run_bass_kernel_spmd(
        _NC, in_maps, core_ids=list(range(NCORES)), **spmd_kwargs
    )
    parts = [res.results[c]["out"] for c in range(NCORES)]
    out = np.empty((2, S, D), dtype=np.float32)
    for b in range(2):
        out[b] = parts[4 * b] + parts[4 * b + 1] + parts[4 * b + 2] + parts[4 * b + 3]
    return out, res


def kernel(x, W_q, W_k, W_v, W_o):
    out, _ = _run(x, W_q, W_k, W_v, W_o)
    return out
